# revision 36
# baseline (speedup 1.0000x reference)
"""Trainium2 Bass kernel for nn_DecoderLayer (self-attn + cross-attn + FFN).

Sharding: 8 cores = (batch b in 0..3) x (query-half in 0..1). Each core
computes 512 query tokens of one batch element end-to-end; K/V projections
over the full source sequence are duplicated across the two halves of a
batch element, so no collectives are needed.

Dtype strategy (rel-err budget 2e-2):
  - fp8(e4m3) + DoubleRow matmuls (2 K-chunks per instruction) for the
    k/v/q projections, scores, the ctx (weights@V) matmul, and the
    attention out-projections. The FFN stays bf16 (fp8 there blows the
    error budget).
  - The additive attention mask is merged INTO the score matmul: one
    DoubleRow instruction whose chunk0 is the K block (the other head in
    the 128-partition pair is nulled by zero-padded Q) and chunk1 is an
    identity against the -240/0 e4m3 mask block. Score psum = 24 x true
    score; exp() applies scale 1/24, bias -4; masked entries reach
    exp(score - 14) -> 0 in e4m3.
  - softmax runs without max-subtraction; the +1s column of V provides
    the denominator. Score psums are 2-bank pairs so each Exp covers
    1024 columns.

Self-contained: hardcodes all shapes; no sibling imports.
"""

import numpy as np
import ml_dtypes
from contextlib import ExitStack

import concourse.bass as bass
import concourse.tile as tile
from concourse import bacc, mybir
from concourse.bass_utils import run_bass_kernel_spmd
from concourse.masks import make_identity

P = 128
LN_EPS = 1e-5

F32 = mybir.dt.float32
BF16 = mybir.dt.bfloat16
FP8 = mybir.dt.float8e4      # e4m3, max normal 240

AF = mybir.ActivationFunctionType
ALU = mybir.AluOpType
DR = mybir.MatmulPerfMode.DoubleRow

# host-side scale folding
QK_SCALE = 1.0 / 24.0        # wq x sqrt(3), wk x sqrt(3) -> scores x24
EXP_BIAS = -4.0              # keeps exp() output inside fp8e4 range
MASK_VAL = -240.0            # e4m3 max; x1/24 => -10 => exp == 0
OUT_SCALE = 1.0 / 32.0       # ctx carries x4 (wv), wo carries x8


def build_decoder_nc(D=1024, S=1024, TP=512, H=16, FF=4096):
    dh = 64
    KC = D // P          # 8 contraction chunks over D
    SB = S // P          # 8 source blocks
    TB = TP // P         # 4 query-token blocks
    NQ = TP              # 512
    VH = 2               # v-proj column halves (512 each)
    VW = D // VH
    ODH = 2              # out-proj column halves
    OW = D // ODH
    FFC = FF // P        # 32
    HPV = VW // dh       # 8 heads per v half
    NU = H + SB          # qzm u-axis: 16 q slots + 8 mask slots

    nc = bacc.Bacc("TRN2", target_bir_lowering=False, debug=False)

    def din(name, shape, dt):
        return nc.dram_tensor(name, shape, dt, kind="ExternalInput").ap()

    xfT8 = din("xfT8", [D, S], FP8)          # x[b]^T (kv source, stage 1)
    xqT8 = din("xqT8", [D, TP], FP8)         # query-slice^T (q source)
    xtok = din("xtok", [TP, D], BF16)        # query-slice (residual)
    encT8 = din("encT8", [D, S], FP8)        # enc_out[b]^T (kv source, st 2)
    qzm1 = din("qzm1", [P, NU - H, NQ], FP8)  # zeros(8) + mask blocks(8)
    qzm2 = din("qzm2", [P, NU - H, NQ], FP8)
    wk1 = din("wk1", [P, KC, KC, P], FP8)    # x sqrt3, of-major lhsT
    wq1 = din("wq1", [P, KC, KC, P], FP8)    # x sqrt3, of-major lhsT
    wv1 = din("wv1", [P, KC, D], FP8)        # x4, moving layout
    wo1 = din("wo1", [P, KC, D], FP8)        # x8, moving layout
    wk2 = din("wk2", [P, KC, KC, P], FP8)
    wq2 = din("wq2", [P, KC, KC, P], FP8)
    wv2 = din("wv2", [P, KC, D], FP8)
    wo2 = din("wo2", [P, KC, D], FP8)
    w8in = din("w8in", [P, FFC, KC, P], BF16)  # per-ffc lhsT chunks
    wout = din("wout", [P, FFC, D], BF16)
    out = nc.dram_tensor("out", [TP, D], F32, kind="ExternalOutput").ap()

    with tile.TileContext(nc) as tc:
        with ExitStack() as ctx:
            consts = ctx.enter_context(tc.tile_pool(name="consts", bufs=1))
            p_stat = ctx.enter_context(tc.tile_pool(name="p_stat", bufs=10))
            p_res = ctx.enter_context(tc.tile_pool(name="p_res", bufs=1))
            p_et = ctx.enter_context(tc.tile_pool(name="p_et", bufs=2))
            pp_pair = ctx.enter_context(
                tc.tile_pool(name="pp_pair", bufs=2, space="PSUM"))
            pp_fill = ctx.enter_context(
                tc.tile_pool(name="pp_fill", bufs=2, space="PSUM"))
            pp_ctx = ctx.enter_context(
                tc.tile_pool(name="pp_ctx", bufs=2, space="PSUM"))

            identf = consts.tile([P, P], F32)
            make_identity(nc, identf)
            ident8 = consts.tile([P, P], FP8)
            nc.gpsimd.tensor_copy(ident8, identf)
            eps_t = consts.tile([P, 1], F32)
            nc.vector.memset(eps_t, LN_EPS)
            ebias_t = consts.tile([P, 1], F32)
            nc.vector.memset(ebias_t, EXP_BIAS)

            # ---------------- helpers -------------------------------------
            def dr_group(ps, wt_of, src, n0, n1):
                """ps = wt_of.T @ src[:, :, n0:n1]; wt_of [P, KC, P]
                of-major weight block, DoubleRow pairs."""
                for kcp in range(KC // 2):
                    nc.tensor.matmul(
                        ps, wt_of[:, 2 * kcp:2 * kcp + 2, :],
                        src[:, 2 * kcp:2 * kcp + 2, n0:n1],
                        start=(kcp == 0), stop=(kcp == KC // 2 - 1),
                        perf_mode=DR)

            def v_group(vt, wvt, kvs, vh, sbg, evict):
                """token-major v projection, 2 source blocks at a time."""
                pss = []
                for sb in (sbg, sbg + 1):
                    ps = pp_fill.tile([P, VW], F32, tag="psf", name="psf")
                    for kcp in range(KC // 2):
                        nc.tensor.matmul(
                            ps, kvs[:, 2 * kcp:2 * kcp + 2,
                                    sb * P:(sb + 1) * P],
                            wvt[:, 2 * kcp:2 * kcp + 2, vh * VW:(vh + 1) * VW],
                            start=(kcp == 0), stop=(kcp == KC // 2 - 1),
                            perf_mode=DR)
                    pss.append(ps)
                for i, sb in enumerate((sbg, sbg + 1)):
                    evict(vt[:, sb, vh * HPV:(vh + 1) * HPV, 0:dh],
                          pss[i].rearrange("p (h d) -> p h d", d=dh))

            def q_proj(wt, src, qzm, of, evict):
                """one 'of' column block -> single-bank psum -> zero-padded
                per-head slots of qzm (heads 2of, 2of+1)."""
                ps = pp_fill.tile([P, NQ], F32, tag="psf", name="psf")
                dr_group(ps, wt[:, of], src, 0, NQ)
                evict(qzm[0:64, 2 * of, :], ps[0:64])
                evict(qzm[64:128, 2 * of + 1, :], ps[64:128])

            def k_proj_half(wt, src, kTI, of, j, evict):
                """single-bank psum = col block 'of', source half j."""
                ps = pp_fill.tile([P, 512], F32, tag="psf", name="psf")
                dr_group(ps, wt[:, of], src, j * 512, (j + 1) * 512)
                evict(kTI[:, 8 * of + 4 * j:8 * of + 4 * j + 4, :]
                      .rearrange("p a w -> p (a w)"), ps)

            def score_head(kTI, qzm, et, h):
                """merged score+mask: one DR instruction per source block.
                chunk0 = K block (other head nulled by zero-padded q),
                chunk1 = identity @ mask block."""
                kc_h = h // 2
                for sbp in range(SB // 2):
                    ps = pp_pair.tile([P, 2, NQ], F32, tag="psp", name="psp")
                    for j, sb in enumerate((2 * sbp, 2 * sbp + 1)):
                        u0 = 8 * kc_h + sb
                        st = kTI[:, u0::64 - u0, :][:, 0:2, :]
                        mv = qzm[:, h::H + sb - h, :][:, 0:2, :]
                        nc.tensor.matmul(ps[:, j, :], st, mv,
                                         start=True, stop=True,
                                         perf_mode=DR)
                    nc.scalar.activation(
                        et[:, 2 * sbp:2 * sbp + 2, :], ps, AF.Exp,
                        bias=ebias_t, scale=QK_SCALE)

            def ctx_head(et, vt, ctxt, h):
                for tb in range(TB):
                    psc = pp_ctx.tile([P, 512], F32, tag="psc", name="psc")
                    for sbp in range(SB // 2):
                        nc.tensor.matmul(
                            psc[:, 0:dh + 1],
                            et[:, 2 * sbp:2 * sbp + 2, tb * P:(tb + 1) * P],
                            vt[:, 2 * sbp:2 * sbp + 2, h, :],
                            start=(sbp == 0), stop=(sbp == SB // 2 - 1),
                            perf_mode=DR)
                    rec = p_stat.tile([P, 1], F32, tag="rec", name="rec")
                    nc.vector.reciprocal(rec, psc[:, dh:dh + 1])
                    nc.vector.tensor_scalar_mul(
                        ctxt[:, tb, h * dh:(h + 1) * dh],
                        in0=psc[:, 0:dh], scalar1=rec)

            def transpose_block(src, dstT, tb, fc, act=False):
                fp8_in = src.dtype == FP8
                ps = pp_fill.tile([P, P], FP8 if fp8_in else BF16,
                                  tag="psf", name="psf")
                nc.tensor.transpose(ps, src[:, tb, fc * P:(fc + 1) * P],
                                    ident8 if fp8_in else identb)
                if act:
                    nc.scalar.copy(dstT[:, fc, tb * P:(tb + 1) * P], ps)
                else:
                    nc.vector.tensor_copy(dstT[:, fc, tb * P:(tb + 1) * P],
                                          ps)

            identb = consts.tile([P, P], BF16)
            nc.gpsimd.tensor_copy(identb, identf)

            def alloc_8psums():
                """8 [P, OW] f32 psums: 2 pair tiles (as 4 halves) + 2 fill
                + 2 ctx."""
                pss = {}
                pairs = [pp_pair.tile([P, 2, OW], F32, tag="psp", name="psp")
                         for _ in range(2)]
                pss[(0, 0)] = pairs[0][:, 0, :]
                pss[(0, 1)] = pairs[0][:, 1, :]
                pss[(0, 2)] = pairs[1][:, 0, :]
                pss[(0, 3)] = pairs[1][:, 1, :]
                pss[(1, 0)] = pp_fill.tile([P, OW], F32, tag="psf",
                                           name="psf")
                pss[(1, 1)] = pp_fill.tile([P, OW], F32, tag="psf",
                                           name="psf")
                pss[(1, 2)] = pp_ctx.tile([P, OW], F32, tag="psc",
                                          name="psc")
                pss[(1, 3)] = pp_ctx.tile([P, OW], F32, tag="psc",
                                          name="psc")
                return pss

            def out_proj_residual(ctxT, wo_t, rsrc, res, per_tb=None):
                """res = ctxT.T @ wo * OUT_SCALE + rsrc; fp8 DoubleRow.
                Token-block-major so each block's residual (and optional
                per_tb(tb) tail: LN + transposes) overlaps the next
                block's matmuls."""
                pss = alloc_8psums()
                for tbp in range(TB // 2):
                    tbs = (2 * tbp, 2 * tbp + 1)
                    for tb in tbs:
                        for oh in range(ODH):
                            for kcp in range(KC // 2):
                                nc.tensor.matmul(
                                    pss[(oh, tb)],
                                    ctxT[:, 2 * kcp:2 * kcp + 2,
                                         tb * P:(tb + 1) * P],
                                    wo_t[:, 2 * kcp:2 * kcp + 2,
                                         oh * OW:(oh + 1) * OW],
                                    start=(kcp == 0),
                                    stop=(kcp == KC // 2 - 1),
                                    perf_mode=DR)
                    for tb in tbs:
                        for oh in range(ODH):
                            nc.vector.scalar_tensor_tensor(
                                out=res[:, tb, oh * OW:(oh + 1) * OW],
                                in0=pss[(oh, tb)], scalar=OUT_SCALE,
                                in1=rsrc[:, tb, oh * OW:(oh + 1) * OW],
                                op0=ALU.mult, op1=ALU.add)
                    if per_tb is not None:
                        for tb in tbs:
                            per_tb(tb)

            def layernorm_tb(res, xout, tb):
                """LN stats on DVE, sqrt on Act, final affine on Pool."""
                st = p_stat.tile([P, 2, 6], F32, tag="lnst", name="lnst")
                for g in range(2):
                    nc.vector.bn_stats(st[:, g, :],
                                       res[:, tb, g * 512:(g + 1) * 512])
                mv = p_stat.tile([P, 2], F32, tag="lnmv", name="lnmv")
                nc.vector.bn_aggr(mv, st)
                std = p_stat.tile([P, 1], F32, tag="lnstd", name="lnstd")
                nc.scalar.activation(std, mv[:, 1:2], AF.Sqrt, bias=eps_t)
                rstd = p_stat.tile([P, 1], F32, tag="lnrstd", name="lnrstd")
                nc.vector.reciprocal(rstd, std)
                nc.gpsimd.tensor_scalar(
                    out=xout[:, tb, :], in0=res[:, tb, :],
                    scalar1=mv[:, 0:1], scalar2=rstd,
                    op0=ALU.subtract, op1=ALU.mult)

            def act_evict(dst, ps):
                nc.scalar.copy(dst, ps)

            def pool_evict(dst, ps):
                # GPSIMD cannot read PSUM on HW; DVE carries in-phase evicts
                nc.vector.tensor_copy(dst, ps)

            # residual-chain tiles (outer, tag-rotated)
            xtok_t = p_res.tile([P, TB, D], BF16, name="xtok_t", bufs=1)
            ctxt1 = p_res.tile([P, TB, D], BF16, tag="ctxt", name="ctxt",
                               bufs=1)
            ctxT1 = p_res.tile([P, KC, TP], FP8, tag="ctxT", name="ctxT",
                               bufs=1)

            pC = ctx.enter_context(tc.tile_pool(name="pC", bufs=1))
            with tc.tile_pool(name="pB", bufs=1) as pB:
                with tc.tile_pool(name="pA", bufs=1) as pA:
                    # q-proj inputs first: smallest DMA set before first matmul
                    qs1 = pA.tile([P, KC, NQ], FP8, name="qs1")
                    nc.sync.dma_start(
                        qs1, xqT8.rearrange("(kc p) t -> p kc t", p=P))
                    wq1t = pA.tile([P, KC, KC, P], FP8, name="wq1t")
                    kvs1 = pA.tile([P, KC, S], FP8, name="kvs1")
                    wk1t = pA.tile([P, KC, KC, P], FP8, name="wk1t")
                    xfr = xfT8.rearrange("(kc p) s -> p kc s", p=P)
                    nc.sync.dma_start(wq1t[:, 0:2], wq1[:, 0:2])
                    nc.sync.dma_start(kvs1[:, :, 0:512], xfr[:, :, 0:512])
                    nc.sync.dma_start(wk1t[:, 0:2], wk1[:, 0:2])
                    nc.sync.dma_start(kvs1[:, :, 512:1024],
                                      xfr[:, :, 512:1024])
                    for c in range(1, 4):
                        nc.sync.dma_start(wq1t[:, 2 * c:2 * c + 2],
                                          wq1[:, 2 * c:2 * c + 2])
                        nc.sync.dma_start(wk1t[:, 2 * c:2 * c + 2],
                                          wk1[:, 2 * c:2 * c + 2])
                    qzm1t = pA.tile([P, NU, NQ], FP8, name="qzm1t")
                    nc.sync.dma_start(qzm1t[:, H:NU, :], qzm1)
                    # zero the unused head-halves of the q slots (the score
                    # matmul reads all 128 partitions of each slot)
                    nc.gpsimd.memset(qzm1t[64:128, 0:H:2, :], 0.0)
                    nc.gpsimd.memset(qzm1t[0:64, 1:H:2, :], 0.0)
                    wv1t = pA.tile([P, KC, D], FP8, name="wv1t")
                    nc.sync.dma_start(wv1t, wv1)
                    kvs2 = pB.tile([P, KC, S], FP8, name="kvs2")
                    nc.sync.dma_start(
                        kvs2, encT8.rearrange("(kc p) s -> p kc s", p=P))
                    wk2t = pB.tile([P, KC, KC, P], FP8, name="wk2t")
                    nc.sync.dma_start(wk2t, wk2)
                    wv2t = pB.tile([P, KC, D], FP8, name="wv2t")
                    nc.sync.dma_start(wv2t, wv2)
                    nc.sync.dma_start(
                        xtok_t, xtok.rearrange("(tb p) d -> p tb d", p=P))
                    wo1t = pB.tile([P, KC, D], FP8, name="wo1t")
                    nc.sync.dma_start(wo1t, wo1)

                    # ---- stage 1: projections interleave into the score
                    # phase as PE fillers (the phase is Act/exp-bound) ----
                    kTI1 = pA.tile([P, 65, P], FP8, name="kTI1")
                    nc.gpsimd.tensor_copy(kTI1[:, 64, :], identf)
                    vt1 = pA.tile([P, SB, H, dh + 1], FP8, name="vt1")
                    nc.gpsimd.memset(vt1[:, :, :, dh:dh + 1], 1.0)
                    kTI2 = pB.tile([P, 65, P], FP8, name="kTI2")
                    nc.gpsimd.tensor_copy(kTI2[:, 64, :], identf)
                    vt2 = pB.tile([P, SB, H, dh + 1], FP8, name="vt2")
                    nc.gpsimd.memset(vt2[:, :, :, dh:dh + 1], 1.0)

                    k_done = set()
                    q_done = set()

                    def need_kq(h):
                        # emit the k/q projections head h's scores read
                        of = h // 2
                        if of not in q_done:
                            q_proj(wq1t, qs1, qzm1t, of, pool_evict)
                            q_done.add(of)
                        if of not in k_done:
                            for j in range(2):
                                k_proj_half(wk1t, kvs1, kTI1, of, j,
                                            pool_evict)
                            k_done.add(of)

                    # filler units (emitted between score heads, in order):
                    # v1 groups (coverage: vh0 before ctx 0, vh1 before
                    # ctx 8), k2 halves, v2 groups.
                    fillers = (
                        [("v1", 0, sbg) for sbg in range(0, SB, 2)]
                        + [("v1", 1, sbg) for sbg in range(0, SB, 2)]
                        + [("k2", of, j) for of in range(KC)
                           for j in range(2)])

                    def run_filler(f):
                        kind, a, b = f
                        if kind == "v1":
                            v_group(vt1, wv1t, kvs1, a, b, pool_evict)
                        else:
                            k_proj_half(wk2t, kvs2, kTI2, a, b, pool_evict)

                    # per-head filler quota: front-load v1 coverage, then
                    # spread the rest across the whole phase
                    quota = {0: 2, 1: 2, 2: 2, 3: 2, 4: 2, 5: 2, 6: 2,
                             7: 2, 8: 1, 9: 1, 10: 1, 11: 1, 12: 1,
                             13: 1, 14: 1, 15: 1}
                    fi = 0

                    ets = {}
                    for h in range(H):
                        need_kq(h)
                        if h + 1 < H:
                            need_kq(h + 1)   # lookahead keeps PE fed
                        ets[h] = p_et.tile([P, SB, NQ], FP8, tag="et",
                                           name="et")
                        score_head(kTI1, qzm1t, ets[h], h)
                        for _ in range(quota.get(h, 0)):
                            if fi < len(fillers):
                                run_filler(fillers[fi])
                                fi += 1
                        if h >= 1:
                            ctx_head(ets[h - 1], vt1, ctxt1, h - 1)
                            ets.pop(h - 1)
                            if (h - 1) % 2 == 1:
                                for tb in range(TB):
                                    transpose_block(ctxt1, ctxT1, tb,
                                                    (h - 1) // 2)
                    ctx_head(ets[H - 1], vt1, ctxt1, H - 1)
                    ets.clear()
                    for tb in range(TB):
                        transpose_block(ctxt1, ctxT1, tb, (H - 1) // 2)
                    while fi < len(fillers):
                        run_filler(fillers[fi])
                        fi += 1

                # pA closed: stage-1 k/q/v tiles + sources freed;
                # pD time-shares that space for stage-2-only tiles
                ctx2 = ctx.enter_context(tc.tile_pool(name="pD", bufs=1))
                pD = ctx2
                wq2t = pD.tile([P, KC, KC, P], FP8, name="wq2t")
                nc.sync.dma_start(wq2t, wq2)
                qzm2t = pD.tile([P, NU, NQ], FP8, name="qzm2t")
                nc.sync.dma_start(qzm2t[:, H:NU, :], qzm2)
                nc.gpsimd.memset(qzm2t[64:128, 0:H:2, :], 0.0)
                nc.gpsimd.memset(qzm2t[0:64, 1:H:2, :], 0.0)
                wo2t = pD.tile([P, KC, D], FP8, name="wo2t")
                nc.sync.dma_start(wo2t, wo2)

                # ---- stage 1 out-proj + LN (pipelined per token block) --
                res1 = p_res.tile([P, TB, D], BF16, tag="rx", name="rx",
                                  bufs=3)
                x1 = p_res.tile([P, TB, D], BF16, tag="rx", name="rx",
                                bufs=3)
                x1T8 = pD.tile([P, KC, TP], FP8, name="x1T8")

                def tail1(tb):
                    layernorm_tb(res1, x1, tb)
                    for fc in range(KC):
                        transpose_block(x1, x1T8, tb, fc)

                out_proj_residual(ctxT1, wo1t, xtok_t, res1, per_tb=tail1)

                # ---- stage 2 -------------------------------------------
                for of in range(KC):
                    q_proj(wq2t, x1T8, qzm2t, of, act_evict)

                ctxt2 = p_res.tile([P, TB, D], BF16, tag="ctxt", name="ctxt",
                                   bufs=1)
                ctxT2 = p_res.tile([P, KC, TP], FP8, tag="ctxT",
                                   name="ctxT", bufs=1)
                ets2 = {}
                for h in range(H):
                    ets2[h] = p_et.tile([P, SB, NQ], FP8, tag="et",
                                        name="et")
                    score_head(kTI2, qzm2t, ets2[h], h)
                    if h >= 1:
                        ctx_head(ets2[h - 1], vt2, ctxt2, h - 1)
                        ets2.pop(h - 1)
                        if (h - 1) % 2 == 1:
                            for tb in range(TB):
                                transpose_block(ctxt2, ctxT2, tb,
                                                (h - 1) // 2)
                ctx_head(ets2[H - 1], vt2, ctxt2, H - 1)
                ets2.clear()
                for tb in range(TB):
                    transpose_block(ctxt2, ctxT2, tb, (H - 1) // 2)

                res2 = p_res.tile([P, TB, D], BF16, tag="rx", name="rx",
                                  bufs=3)
                x2 = p_res.tile([P, TB, D], BF16, tag="rx", name="rx",
                                bufs=3)
                x2T8 = pC.tile([P, KC, TP], BF16, name="x2T8")

                def tail2(tb):
                    layernorm_tb(res2, x2, tb)
                    for fc in range(KC):
                        transpose_block(x2, x2T8, tb, fc)

                out_proj_residual(ctxT2, wo2t, x1, res2, per_tb=tail2)

            # pB closed: stage-2 tiles freed
            # ---- FFN ---------------------------------------------------
            p_hT = ctx.enter_context(tc.tile_pool(name="p_hT", bufs=1))
            hT = p_hT.tile([P, FFC, NQ], BF16, name="hT")
            p_wout = ctx.enter_context(tc.tile_pool(name="p_wout", bufs=3))
            wqts = []
            for q in range(3):
                wqt = p_wout.tile([P, 4, D], BF16, tag="wout", name="wout")
                nc.sync.dma_start(wqt, wout[:, 4 * q:4 * q + 4, :])
                wqts.append(wqt)

            with tc.tile_pool(name="p_win", bufs=4) as p_win:
                wps = []
                for fp in range(3):
                    wp = p_win.tile([P, 2, KC, P], BF16, tag="win",
                                    name="win")
                    nc.sync.dma_start(wp, w8in[:, 2 * fp:2 * fp + 2, :, :])
                    wps.append(wp)
                for fp in range(FFC // 2):
                    if fp + 3 < FFC // 2:
                        wp = p_win.tile([P, 2, KC, P], BF16, tag="win",
                                        name="win")
                        nc.sync.dma_start(
                            wp, w8in[:, 2 * fp + 6:2 * fp + 8, :, :])
                        wps.append(wp)
                    ps = pp_pair.tile([P, 2, NQ], F32, tag="psp",
                                      name="psp")
                    for f in range(2):
                        ffc = 2 * fp + f
                        for kc in range(KC):
                            nc.tensor.matmul(
                                ps[:, f, :],
                                wps[fp][:, f, kc, :],
                                x2T8[:, kc, :],
                                start=(kc == 0), stop=(kc == KC - 1))
                    nc.scalar.activation(hT[:, 2 * fp:2 * fp + 2, :], ps,
                                         AF.Relu)

            # FFN second layer: ffc 0..23 accumulate for all groups, then
            # per token block ffc 24..31 + residual + LN + store so each
            # block's tail overlaps the next block's matmuls.
            res3 = p_res.tile([P, TB, D], BF16, tag="rx", name="rx",
                              bufs=3)
            pss = alloc_8psums()
            for q in range(6):
                if q >= 3:
                    wqt = p_wout.tile([P, 4, D], BF16, tag="wout",
                                      name="wout")
                    nc.sync.dma_start(wqt, wout[:, 4 * q:4 * q + 4, :])
                    wqts.append(wqt)
                for f in range(4):
                    ffc = 4 * q + f
                    for oh in range(ODH):
                        for tb in range(TB):
                            nc.tensor.matmul(
                                pss[(oh, tb)],
                                hT[:, ffc, tb * P:(tb + 1) * P],
                                wqts[q][:, f, oh * OW:(oh + 1) * OW],
                                start=(ffc == 0), stop=False)
            for q in (6, 7):
                wqt = p_wout.tile([P, 4, D], BF16, tag="wout", name="wout")
                nc.sync.dma_start(wqt, wout[:, 4 * q:4 * q + 4, :])
                wqts.append(wqt)

            outr = out.rearrange("(tb p) d -> p tb d", p=P)

            def finish_tb(tb):
                for oh in range(ODH):
                    nc.vector.tensor_tensor(
                        res3[:, tb, oh * OW:(oh + 1) * OW], pss[(oh, tb)],
                        x2[:, tb, oh * OW:(oh + 1) * OW], ALU.add)
                st = p_stat.tile([P, 2, 6], F32, tag="lnst", name="lnst")
                for g in range(2):
                    nc.vector.bn_stats(st[:, g, :],
                                       res3[:, tb, g * 512:(g + 1) * 512])
                mv = p_stat.tile([P, 2], F32, tag="lnmv", name="lnmv")
                nc.vector.bn_aggr(mv, st)
                std = p_stat.tile([P, 1], F32, tag="lnstd", name="lnstd")
                nc.scalar.activation(std, mv[:, 1:2], AF.Sqrt, bias=eps_t)
                rstd = p_stat.tile([P, 1], F32, tag="lnrstd", name="lnrstd")
                nc.vector.reciprocal(rstd, std)
                xo = p_res.tile([P, D], F32, tag="xo", name="xo", bufs=2)
                nc.gpsimd.tensor_scalar(
                    out=xo, in0=res3[:, tb, :],
                    scalar1=mv[:, 0:1], scalar2=rstd,
                    op0=ALU.subtract, op1=ALU.mult)
                nc.sync.dma_start(outr[:, tb, :], xo)

            for tbp in range(TB // 2):
                for tb in (2 * tbp, 2 * tbp + 1):
                    for q in (6, 7):
                        for f in range(4):
                            ffc = 4 * q + f
                            for oh in range(ODH):
                                nc.tensor.matmul(
                                    pss[(oh, tb)],
                                    hT[:, ffc, tb * P:(tb + 1) * P],
                                    wqts[q][:, f, oh * OW:(oh + 1) * OW],
                                    start=False, stop=(ffc == FFC - 1))
                for tb in (2 * tbp, 2 * tbp + 1):
                    finish_tb(tb)

    nc.compile()
    return nc


# ---------------------------------------------------------------------------
# host side
# ---------------------------------------------------------------------------

_NC_CACHE = {}


def _get_nc(key="v4"):
    if key not in _NC_CACHE:
        _NC_CACHE[key] = build_decoder_nc()
    return _NC_CACHE[key]


MM_KEY = "v4"

E4 = ml_dtypes.float8_e4m3
BF = ml_dtypes.bfloat16
SQ3 = np.float32(np.sqrt(3.0))


def _lhsT_layout(w):
    """[D, M] -> [P, D//P, M] (row chunks onto partitions)."""
    Dd, M = w.shape
    return np.ascontiguousarray(
        w.reshape(Dd // P, P, M).transpose(1, 0, 2))


def _ofm_layout(w):
    """[D, M] -> [P, M//P, D//P, P] (of-major lhsT blocks)."""
    Dd, M = w.shape
    return np.ascontiguousarray(
        w.reshape(Dd // P, P, M // P, P).transpose(1, 2, 0, 3))


def _numpy_reference(x, enc_out, src_mask, tgt_mask, wq1, bq1, wkv1, bkv1,
                     wo1, bo1, wq2, bq2, wkv2, bkv2, wo2, bo2, w_in, b_in,
                     w_out, b_out, g0, be0, g1, be1, g2, be2):
    """Pure-numpy fallback (exact reference semantics)."""
    H, D = 16, 1024

    def ln(x, g, b):
        m = x.mean(-1, keepdims=True)
        v = ((x - m) ** 2).mean(-1, keepdims=True)
        return (x - m) / np.sqrt(v + LN_EPS) * g + b

    def attn(q_in, mem, mask, wq, bq, wkv, bkv, wo, bo):
        B, T, _ = q_in.shape
        S = mem.shape[1]
        dhl = D // H
        q = (q_in @ wq + bq).reshape(B, T, H, dhl) * (dhl ** -0.5)
        k, v = np.split(mem @ wkv + bkv, 2, axis=-1)
        k = k.reshape(B, S, H, dhl)
        v = v.reshape(B, S, H, dhl)
        sc = np.einsum('bthd,bshd->bhts', q, k)
        sc = np.where(mask[:, None, :, :], -1e20, sc)
        sc = sc - sc.max(-1, keepdims=True)
        w = np.exp(sc)
        w = w / w.sum(-1, keepdims=True)
        ctx = np.einsum('bhts,bshd->bthd', w, v).reshape(B, T, D)
        return ctx @ wo + bo

    y = attn(x, x, tgt_mask, wq1, bq1, wkv1, bkv1, wo1, bo1)
    x1 = ln(x + y, g0, be0)
    y = attn(x1, enc_out, src_mask, wq2, bq2, wkv2, bkv2, wo2, bo2)
    x2 = ln(x1 + y, g1, be1)
    y = np.maximum(x2 @ w_in + b_in, 0.0) @ w_out + b_out
    return ln(x2 + y, g2, be2)


def kernel(x, enc_out, src_mask, tgt_mask, wq1, bq1, wkv1, bkv1, wo1, bo1,
           wq2, bq2, wkv2, bkv2, wo2, bo2, w_in, b_in, w_out, b_out,
           g0, be0, g1, be1, g2, be2, _trace=False):
    x = np.asarray(x)
    args = dict(x=x, enc_out=np.asarray(enc_out),
                src_mask=np.asarray(src_mask), tgt_mask=np.asarray(tgt_mask),
                wq1=np.asarray(wq1), bq1=np.asarray(bq1),
                wkv1=np.asarray(wkv1), bkv1=np.asarray(bkv1),
                wo1=np.asarray(wo1), bo1=np.asarray(bo1),
                wq2=np.asarray(wq2), bq2=np.asarray(bq2),
                wkv2=np.asarray(wkv2), bkv2=np.asarray(bkv2),
                wo2=np.asarray(wo2), bo2=np.asarray(bo2),
                w_in=np.asarray(w_in), b_in=np.asarray(b_in),
                w_out=np.asarray(w_out), b_out=np.asarray(b_out),
                g0=np.asarray(g0), be0=np.asarray(be0),
                g1=np.asarray(g1), be1=np.asarray(be1),
                g2=np.asarray(g2), be2=np.asarray(be2))

    # the hardware kernel folds out zero biases / unit gains (true for this
    # problem's setup_inputs); anything else falls back to exact numpy.
    zeros = [args[k] for k in ("bq1", "bkv1", "bo1", "bq2", "bkv2", "bo2",
                               "b_in", "b_out", "be0", "be1", "be2")]
    ones = [args["g0"], args["g1"], args["g2"]]
    if any(np.any(z != 0) for z in zeros) or any(np.any(g != 1) for g in ones):
        res = _numpy_reference(**args)
        return res.astype(np.float32), x

    B, T, D = x.shape
    TP = T // 2

    # shared weight conversions (lhsT layouts + fp8 scale folding)
    wk_1 = _ofm_layout(args["wkv1"][:, :D] * SQ3).astype(E4)
    wv_1 = _lhsT_layout(args["wkv1"][:, D:] * 4.0).astype(E4)
    wq_1 = _ofm_layout(args["wq1"] * SQ3).astype(E4)
    wo_1 = _lhsT_layout(args["wo1"] * 8.0).astype(E4)
    wk_2 = _ofm_layout(args["wkv2"][:, :D] * SQ3).astype(E4)
    wv_2 = _lhsT_layout(args["wkv2"][:, D:] * 4.0).astype(E4)
    wq_2 = _ofm_layout(args["wq2"] * SQ3).astype(E4)
    wo_2 = _lhsT_layout(args["wo2"] * 8.0).astype(E4)
    KC, FFC = D // P, args["w_in"].shape[1] // P
    w8in = np.ascontiguousarray(
        args["w_in"].reshape(KC, P, FFC, P)
        .transpose(1, 2, 0, 3)).astype(BF)
    wout = _lhsT_layout(args["w_out"]).astype(BF)

    SB, NQ = T // P, TP

    def mk_qzm(mask_slice):
        """[TP, S] bool -> [P, SB, NQ] e4m3 additive mask blocks
        (transposed s-major)."""
        S = mask_slice.shape[1]
        mT = np.where(mask_slice.T, np.float32(MASK_VAL), np.float32(0.0))
        return np.ascontiguousarray(
            mT.reshape(S // P, P, NQ).transpose(1, 0, 2)).astype(E4)

    in_maps = []
    for core in range(8):
        b, half = divmod(core, 2)
        t0 = half * TP
        xb = args["x"][b]
        xs = xb[t0:t0 + TP]
        in_maps.append({
            "xfT8": np.ascontiguousarray(xb.T).astype(E4),
            "xqT8": np.ascontiguousarray(xs.T).astype(E4),
            "xtok": np.ascontiguousarray(xs).astype(BF),
            "encT8": np.ascontiguousarray(args["enc_out"][b].T).astype(E4),
            "qzm1": mk_qzm(args["tgt_mask"][b, t0:t0 + TP]),
            "qzm2": mk_qzm(args["src_mask"][b, t0:t0 + TP]),
            "wk1": wk_1, "wq1": wq_1, "wv1": wv_1, "wo1": wo_1,
            "wk2": wk_2, "wq2": wq_2, "wv2": wv_2, "wo2": wo_2,
            "w8in": w8in, "wout": wout,
        })

    nc = _get_nc(MM_KEY)
    res = run_bass_kernel_spmd(nc, in_maps, core_ids=list(range(8)),
                               trace=_trace)
    outp = np.empty((B, T, D), np.float32)
    for core in range(8):
        b, half = divmod(core, 2)
        outp[b, half * TP:(half + 1) * TP] = res.results[core]["out"]
    if _trace:
        kernel.last_results = res
    return outp, x
